# revision 12
# baseline (speedup 1.0000x reference)
"""DynamicSparseMoE grouped-GEMM kernel for 8 TRN2 NeuronCores.

out[t] = tokens[t] @ weight[exp_ids[t]]   (T=8192, E=8, D=2048 -> 2048)

Strategy (expert-parallel, host-side dispatch):
  - Host sorts tokens by expert; core e owns expert e's weight and its
    routed tokens, padded to a common capacity C (SPMD needs equal shapes).
  - Inputs are cast to fp16 on the host (PE runs fp16 at 1 cyc/row vs
    fp32's 4; PSUM accumulation stays fp32, measured rel-err ~3e-4).
  - Tokens are passed transposed ([D, C]): the stationary operand is a
    token block xT[d-block, 128 t] (one LDWEIGHTS per 4 matmuls), the
    moving operand is a weight slice w[d-block, 512 o], and PSUM gets
    out[t-block, o-slice] in the natural output orientation.
  - t-blocks are processed in pairs with the contraction (kb) loop
    outermost inside the pair: 8 PSUM banks hold 2x4 accumulation groups
    and the PE can start as soon as the first kb-block of x/w arrives
    instead of waiting for the whole 8 MB weight.
  - Everything (x, w) is SBUF-resident; out streams per t-block.
"""

import numpy as np

P = 128
D = 2048
E = 8
KB = D // P  # 16 contraction blocks
NOS = 4  # 4 moving slices of 512 over the 2048 output dim
NS = D // NOS  # 512

_cache = {}


def _ensure_imports():
    try:
        import concourse.bass  # noqa: F401
    except ImportError:
        import sys

        for p in ("/opt/trn_rl_repo", "/opt/pypackages"):
            if p not in sys.path:
                sys.path.append(p)


def _np_dt(compute_dt):
    if compute_dt == "float16":
        return np.float16
    import ml_dtypes

    return ml_dtypes.bfloat16


def _build(C, compute_dt="float16"):
    """Build + compile the per-core Bass program for capacity C."""
    _ensure_imports()
    import concourse.bacc as bacc
    import concourse.mybir as mybir
    import concourse.tile as tile

    cdt = getattr(mybir.dt, compute_dt)
    TB = C // P  # t-blocks

    nc = bacc.Bacc(None, target_bir_lowering=False, debug=False)
    n0 = 2 * P if C // P >= 2 else P
    xt0_d = nc.declare_dram_parameter("xt0", [P, KB * n0], cdt, isOutput=False)
    xt_d = nc.declare_dram_parameter("xt", [D, C], cdt, isOutput=False)
    w_d = nc.declare_dram_parameter("w", [D, D], cdt, isOutput=False)
    out_d = nc.declare_dram_parameter("out", [C, D], mybir.dt.float32, isOutput=True)

    xt_t = xt_d.rearrange("(k p) n -> p k n", p=P)  # [128, 16, C]
    w_t = w_d.rearrange("(k p) o -> p k o", p=P)  # [128, 16, 2048]

    pairs = [[tb for tb in (p0, p0 + 1) if tb < TB] for p0 in range(0, TB, 2)]

    with tile.TileContext(nc) as tc:
        with (
            tc.tile_pool(name="wp", bufs=1) as wp,
            tc.tile_pool(name="xp", bufs=1) as xp,
            tc.tile_pool(name="op", bufs=3) as op,
            tc.tile_pool(name="pp", bufs=8, space="PSUM") as pp,
        ):
            # Pair 0's stationary blocks come from a dedicated host-packed
            # contiguous tile loaded before the 8 MB weight stream; the
            # remaining x arrives after the weights, by which time pairs 1+
            # still lead the PE comfortably. The very first matmul only
            # needs xp0's first kb-blocks and w0's first o-slice, so those
            # land as small separate DMAs ahead of everything else.
            N0A = 2  # kb blocks in the first x chunk
            xp0a = xp.tile([P, N0A * n0], cdt, tag="xp0a")
            nc.sync.dma_start(xp0a[:], xt0_d[:, : N0A * n0])
            w0 = wp.tile([P, D], cdt, tag="w0")
            nc.sync.dma_start(w0[:, :NS], w_t[:, 0, :NS])
            nc.sync.dma_start(w0[:, NS:], w_t[:, 0, NS:])
            w1 = wp.tile([P, D], cdt, tag="w1")
            nc.sync.dma_start(w1[:], w_t[:, 1, :])
            xp0b = xp.tile([P, (KB - N0A) * n0], cdt, tag="xp0b")
            nc.sync.dma_start(xp0b[:], xt0_d[:, N0A * n0 :])

            def xp0(kb):
                if kb < N0A:
                    return xp0a[:, kb * n0 : (kb + 1) * n0]
                return xp0b[:, (kb - N0A) * n0 : (kb - N0A + 1) * n0]

            w_sb = [w0, w1]
            for kb in range(2, KB):
                w_k = wp.tile([P, D], cdt, tag=f"w{kb}")
                nc.sync.dma_start(w_k[:], w_t[:, kb, :])
                w_sb.append(w_k)
            x_sb = []
            for kb in range(KB):
                xt_k = xp.tile([P, C], cdt, tag=f"x{kb}")
                nc.sync.dma_start(xt_k[:], xt_t[:, kb, :])
                x_sb.append(xt_k)

            def lhs(pi, kb, tb, ti, ntb):
                if pi == 0:
                    return xp0(kb)[:, ti * P : (ti + 1) * P]
                return x_sb[kb][:, tb * P : (tb + 1) * P]

            # PE pre-warm: HAM keeps the PE clock-gated at 1.2 GHz until it
            # has seen ~3.4 us of sustained activity. Run dummy matmuls on
            # memset data during the initial DMA wait so the real matmuls
            # start at 2.4 GHz. They scribble on pair 0's first PSUM bank,
            # which the first real start=True matmul clears anyway.
            warm = xp.tile([P, 64], cdt, tag="warm")
            nc.gpsimd.memset(warm[:], 0.0)

            for pi, tbs in enumerate(pairs):
                last = pi == len(pairs) - 1
                ps = {
                    (ti, os): pp.tile(
                        [P, NS], mybir.dt.float32, tag="ps", name=f"ps_{pi}_{ti}_{os}"
                    )
                    for ti in range(len(tbs))
                    for os in range(NOS)
                }
                if pi == 0:
                    for _ in range(56):
                        nc.tensor.matmul(
                            ps[(0, 0)][:64, :64],
                            lhsT=warm[:, :64],
                            rhs=warm[:, :64],
                            start=True,
                            stop=True,
                        )
                for kb in range(KB):
                    for ti, tb in enumerate(tbs):
                        for os in range(NOS):
                            nc.tensor.matmul(
                                ps[(ti, os)][:],
                                lhsT=lhs(pi, kb, tb, ti, len(tbs)),
                                rhs=w_sb[kb][:, os * NS : (os + 1) * NS],
                                start=(kb == 0),
                                stop=(kb == KB - 1),
                            )
                for ti, tb in enumerate(tbs):
                    o_sb = op.tile([P, D], mybir.dt.float32, tag="o", name=f"o_{pi}_{ti}")
                    for os in range(NOS):
                        nc.vector.tensor_copy(
                            o_sb[:, os * NS : (os + 1) * NS], ps[(ti, os)][:]
                        )
                        if last:
                            # tail: stream each 512-slice out as soon as its
                            # copy lands instead of one 1 MB DMA at the end
                            nc.scalar.dma_start(
                                out_d[tb * P : (tb + 1) * P, os * NS : (os + 1) * NS],
                                o_sb[:, os * NS : (os + 1) * NS],
                            )
                    if not last:
                        nc.scalar.dma_start(out_d[tb * P : (tb + 1) * P, :], o_sb[:])
    nc.compile()
    return nc


def _get_nc(C, compute_dt):
    key = (C, compute_dt)
    if key not in _cache:
        _cache[key] = _build(C, compute_dt)
    return _cache[key]


def kernel(tokens, weight, exp_ids, _trace=False, _compute_dt="float16"):
    _ensure_imports()
    from concourse.bass_utils import run_bass_kernel_spmd

    tokens = np.asarray(tokens)
    weight = np.asarray(weight)
    exp_ids = np.asarray(exp_ids)
    T = tokens.shape[0]

    order = np.argsort(exp_ids, kind="stable")
    counts = np.bincount(exp_ids, minlength=E)
    C = max(int(-(-counts.max() // P) * P), NS)

    starts = np.zeros(E + 1, dtype=np.int64)
    np.cumsum(counts, out=starts[1:])

    npdt = _np_dt(_compute_dt)
    tokens_c = tokens.astype(npdt)
    weight_c = weight.astype(npdt)

    n0 = 2 * P if C // P >= 2 else P
    in_maps = []
    for e in range(E):
        idx = order[starts[e] : starts[e + 1]]
        xt = np.zeros((D, C), dtype=npdt)
        xt[:, : counts[e]] = tokens_c[idx].T
        # xt0: first-pair stationary blocks packed [p, kb*n0 + t] contiguously
        xt0 = np.ascontiguousarray(
            xt[:, :n0].reshape(KB, P, n0).transpose(1, 0, 2).reshape(P, KB * n0)
        )
        in_maps.append({"xt": xt, "xt0": xt0, "w": np.ascontiguousarray(weight_c[e])})

    nc = _get_nc(C, _compute_dt)
    res = run_bass_kernel_spmd(
        nc,
        in_maps,
        core_ids=list(range(E)),
        trace=_trace,
        trace_cores=list(range(E)) if _trace else None,
    )

    out = np.empty((T, D), dtype=np.float32)
    for e in range(E):
        idx = order[starts[e] : starts[e + 1]]
        out[idx] = res.results[e]["out"][: counts[e], :]
    if _trace:
        return out, res
    return out


# revision 13
# speedup vs baseline: 1.0375x; 1.0375x over previous
"""DynamicSparseMoE grouped-GEMM kernel for 8 TRN2 NeuronCores.

out[t] = tokens[t] @ weight[exp_ids[t]]   (T=8192, E=8, D=2048 -> 2048)

Strategy (expert-parallel, host-side dispatch):
  - Host sorts tokens by expert; core e owns expert e's weight and its
    routed tokens, padded to a common capacity C (SPMD needs equal shapes).
  - Inputs are cast to fp16 on the host (PE runs fp16 at 1 cyc/row vs
    fp32's 4; PSUM accumulation stays fp32, measured rel-err ~3e-4).
  - Tokens are passed transposed ([D, C]): the stationary operand is a
    token block xT[d-block, 128 t] (one LDWEIGHTS per 4 matmuls), the
    moving operand is a weight slice w[d-block, 512 o], and PSUM gets
    out[t-block, o-slice] in the natural output orientation.
  - t-blocks are processed in pairs with the contraction (kb) loop
    outermost inside the pair: 8 PSUM banks hold 2x4 accumulation groups
    and the PE can start as soon as the first kb-block of x/w arrives
    instead of waiting for the whole 8 MB weight.
  - Everything (x, w) is SBUF-resident; out streams per t-block.
"""

import numpy as np

P = 128
D = 2048
E = 8
KB = D // P  # 16 contraction blocks
NOS = 4  # 4 moving slices of 512 over the 2048 output dim
NS = D // NOS  # 512

_cache = {}


def _ensure_imports():
    try:
        import concourse.bass  # noqa: F401
    except ImportError:
        import sys

        for p in ("/opt/trn_rl_repo", "/opt/pypackages"):
            if p not in sys.path:
                sys.path.append(p)


def _np_dt(compute_dt):
    if compute_dt == "float16":
        return np.float16
    import ml_dtypes

    return ml_dtypes.bfloat16


def _build(C, compute_dt="float16"):
    """Build + compile the per-core Bass program for capacity C."""
    _ensure_imports()
    import concourse.bacc as bacc
    import concourse.mybir as mybir
    import concourse.tile as tile

    cdt = getattr(mybir.dt, compute_dt)
    TB = C // P  # t-blocks

    nc = bacc.Bacc(None, target_bir_lowering=False, debug=False)
    n0 = 2 * P if C // P >= 2 else P
    xt0_d = nc.declare_dram_parameter("xt0", [P, KB * n0], cdt, isOutput=False)
    xt_d = nc.declare_dram_parameter("xt", [D, C], cdt, isOutput=False)
    w_d = nc.declare_dram_parameter("w", [D, D], cdt, isOutput=False)
    out_d = nc.declare_dram_parameter("out", [C, D], mybir.dt.float32, isOutput=True)

    xt_t = xt_d.rearrange("(k p) n -> p k n", p=P)  # [128, 16, C]
    w_t = w_d.rearrange("(k p) o -> p k o", p=P)  # [128, 16, 2048]

    pairs = [[tb for tb in (p0, p0 + 1) if tb < TB] for p0 in range(0, TB, 2)]

    with tile.TileContext(nc) as tc:
        with (
            tc.tile_pool(name="wp", bufs=1) as wp,
            tc.tile_pool(name="xp", bufs=1) as xp,
            tc.tile_pool(name="op", bufs=3) as op,
            tc.tile_pool(name="pp", bufs=8, space="PSUM") as pp,
        ):
            # Pair 0's stationary blocks come from a dedicated host-packed
            # contiguous tile loaded before the 8 MB weight stream; the
            # remaining x arrives after the weights, by which time pairs 1+
            # still lead the PE comfortably. The very first matmul only
            # needs xp0's first kb-blocks and w0's first o-slice, so those
            # land as small separate DMAs ahead of everything else.
            N0A = 2  # kb blocks in the first x chunk
            xp0a = xp.tile([P, N0A * n0], cdt, tag="xp0a")
            nc.sync.dma_start(xp0a[:], xt0_d[:, : N0A * n0])
            w0 = wp.tile([P, D], cdt, tag="w0")
            nc.sync.dma_start(w0[:, :NS], w_t[:, 0, :NS])
            nc.sync.dma_start(w0[:, NS:], w_t[:, 0, NS:])
            w1 = wp.tile([P, D], cdt, tag="w1")
            nc.sync.dma_start(w1[:], w_t[:, 1, :])
            xp0b = xp.tile([P, (KB - N0A) * n0], cdt, tag="xp0b")
            nc.sync.dma_start(xp0b[:], xt0_d[:, N0A * n0 :])

            def xp0(kb):
                if kb < N0A:
                    return xp0a[:, kb * n0 : (kb + 1) * n0]
                return xp0b[:, (kb - N0A) * n0 : (kb - N0A + 1) * n0]

            w_sb = [w0, w1]
            for kb in range(2, KB):
                w_k = wp.tile([P, D], cdt, tag=f"w{kb}")
                nc.sync.dma_start(w_k[:], w_t[:, kb, :])
                w_sb.append(w_k)
            x_sb = []
            for kb in range(KB):
                xt_k = xp.tile([P, C], cdt, tag=f"x{kb}")
                nc.sync.dma_start(xt_k[:], xt_t[:, kb, :])
                x_sb.append(xt_k)

            def lhs(pi, kb, tb, ti, ntb):
                if pi == 0:
                    return xp0(kb)[:, ti * P : (ti + 1) * P]
                return x_sb[kb][:, tb * P : (tb + 1) * P]

            # PE pre-warm: HAM keeps the PE clock-gated at 1.2 GHz until it
            # has seen ~3.4 us of sustained activity. Run dummy matmuls on
            # memset data during the initial DMA wait so the real matmuls
            # start at 2.4 GHz. They scribble on pair 0's first PSUM bank,
            # which the first real start=True matmul clears anyway.
            warm = xp.tile([P, 64], cdt, tag="warm")
            nc.gpsimd.memset(warm[:], 0.0)

            for pi, tbs in enumerate(pairs):
                last = pi == len(pairs) - 1
                ps = {
                    (ti, os): pp.tile(
                        [P, NS], mybir.dt.float32, tag="ps", name=f"ps_{pi}_{ti}_{os}"
                    )
                    for ti in range(len(tbs))
                    for os in range(NOS)
                }
                if pi == 0:
                    for _ in range(80):
                        nc.tensor.matmul(
                            ps[(0, 0)][:64, :64],
                            lhsT=warm[:, :64],
                            rhs=warm[:, :64],
                            start=True,
                            stop=True,
                        )
                for kb in range(KB):
                    for ti, tb in enumerate(tbs):
                        for os in range(NOS):
                            nc.tensor.matmul(
                                ps[(ti, os)][:],
                                lhsT=lhs(pi, kb, tb, ti, len(tbs)),
                                rhs=w_sb[kb][:, os * NS : (os + 1) * NS],
                                start=(kb == 0),
                                stop=(kb == KB - 1),
                            )
                for ti, tb in enumerate(tbs):
                    o_sb = op.tile([P, D], mybir.dt.float32, tag="o", name=f"o_{pi}_{ti}")
                    for os in range(NOS):
                        nc.vector.tensor_copy(
                            o_sb[:, os * NS : (os + 1) * NS], ps[(ti, os)][:]
                        )
                        if last:
                            # tail: stream each 512-slice out as soon as its
                            # copy lands instead of one 1 MB DMA at the end
                            nc.scalar.dma_start(
                                out_d[tb * P : (tb + 1) * P, os * NS : (os + 1) * NS],
                                o_sb[:, os * NS : (os + 1) * NS],
                            )
                    if not last:
                        nc.scalar.dma_start(out_d[tb * P : (tb + 1) * P, :], o_sb[:])
    nc.compile()
    return nc


def _get_nc(C, compute_dt):
    key = (C, compute_dt)
    if key not in _cache:
        _cache[key] = _build(C, compute_dt)
    return _cache[key]


def kernel(tokens, weight, exp_ids, _trace=False, _compute_dt="float16"):
    _ensure_imports()
    from concourse.bass_utils import run_bass_kernel_spmd

    tokens = np.asarray(tokens)
    weight = np.asarray(weight)
    exp_ids = np.asarray(exp_ids)
    T = tokens.shape[0]

    order = np.argsort(exp_ids, kind="stable")
    counts = np.bincount(exp_ids, minlength=E)
    C = max(int(-(-counts.max() // P) * P), NS)

    starts = np.zeros(E + 1, dtype=np.int64)
    np.cumsum(counts, out=starts[1:])

    npdt = _np_dt(_compute_dt)
    tokens_c = tokens.astype(npdt)
    weight_c = weight.astype(npdt)

    n0 = 2 * P if C // P >= 2 else P
    in_maps = []
    for e in range(E):
        idx = order[starts[e] : starts[e + 1]]
        xt = np.zeros((D, C), dtype=npdt)
        xt[:, : counts[e]] = tokens_c[idx].T
        # xt0: first-pair stationary blocks packed [p, kb*n0 + t] contiguously
        xt0 = np.ascontiguousarray(
            xt[:, :n0].reshape(KB, P, n0).transpose(1, 0, 2).reshape(P, KB * n0)
        )
        in_maps.append({"xt": xt, "xt0": xt0, "w": np.ascontiguousarray(weight_c[e])})

    nc = _get_nc(C, _compute_dt)
    res = run_bass_kernel_spmd(
        nc,
        in_maps,
        core_ids=list(range(E)),
        trace=_trace,
        trace_cores=list(range(E)) if _trace else None,
    )

    out = np.empty((T, D), dtype=np.float32)
    for e in range(E):
        idx = order[starts[e] : starts[e + 1]]
        out[idx] = res.results[e]["out"][: counts[e], :]
    if _trace:
        return out, res
    return out
